# revision 15
# baseline (speedup 1.0000x reference)
"""DotAttackHead kernel for Trainium2 (8 NeuronCores, data-parallel over batch).

prob = softmax(relu(ufeat @ W.T + b) @ efeat.T / sqrt(256) + mask_bias)
W = g * v / ||v||_F

Sharding: batch 64 -> 8 cores x 8 batches (data-parallel). Params replicated.

Host prep: weight-norm W, bf16 cast, mask folded into efeat (masked columns
poisoned with -1e30 so masked logits underflow exp to exactly 0), and BOTH
inputs staged in the exact SBUF tile layout ([p, kt, u] / [p, et, n]) so
every load is a pure linear 128-partition DMA with 4-8KB contiguous
per-partition chunks (the naive [K, U]-transposing load produces 1KB
descriptors and runs at ~1/4 line rate).

Device per batch b (software-pipelined across batches):
  mm1:  projT[e,u] = relu(wT.T @ ufT[b] + bias)   (PE bf16; bias+relu fused
        on DVE as tensor_scalar add/max reading PSUM, bf16 out)
  mm2:  psum[u,n]  = projT.T @ efT[b]             (PE bf16, fp32 PSUM)
  soft: et = Exp(psum/16) with accum_out row-sum (ACT, bf16, written
        directly into the ganged [128,4,W] store tile). The softmax
        division happens on the HOST during the f32 upcast (prob = et / s),
        which removes the reciprocal+multiply (35us of DVE) from the device.
Row sums gang into one [128, bpc, 8] tile, DMA'd once at the end.

Output is stored in a COMPACT partition-major layout: one flat bf16 tensor,
per (slot, 4-u-tile gang) a [128, 4, W] block written as one linear DMA
(4W*2B contiguous per partition -> max-size descriptors). The host
un-permutes to [U, N], divides by the row sums and zero-fills columns
[W, N). Masked-width specialization: batches sorted by effective width
descending, rank 8k+c -> (core c, slot k), slot width = slot max rounded
up to 128, compiled per-widths (NEFF-cached).

No max-subtraction: logits are O(+-6) so exp is safe in fp32, and softmax is
shift-invariant, so this matches the reference.

fp8 was evaluated for the matmuls (DoubleRow, 2x PE) and REJECTED: e4m3
quantization of proj+efeat pushes rel err to 4e-2 > the 2e-2 gate.
"""

from contextlib import ExitStack

import ml_dtypes
import numpy as np

import concourse.bass as bass
import concourse.mybir as mybir
import concourse.tile as tile
from concourse import bacc
from concourse.bass_utils import run_bass_kernel_spmd

N_CORES = 8
B = 64
U = 1024  # units
E = 256   # efeat dim
K = 512   # ufeat dim
N = 1024  # enemies
BPC = B // N_CORES  # batches per core

F32 = mybir.dt.float32
BF16 = mybir.dt.bfloat16
BF16_NP = ml_dtypes.bfloat16

def _build_bass(bpc: int = BPC, widths: tuple = ()) -> bass.Bass:
    if not widths:
        widths = (N,) * bpc
    assert len(widths) == bpc and all(w % 64 == 0 and 128 <= w <= N for w in widths)
    # Bacc (not raw Bass): its finalize() runs generate_event_semaphores,
    # which splits multi-wait instructions to satisfy TRN2's 1-wait limit.
    nc = bacc.Bacc(None, target_bir_lowering=False)

    # inputs staged host-side in SBUF layout: linear 128-partition loads
    ufT = nc.declare_dram_parameter("ufT", [bpc, 128, 4 * U], BF16, isOutput=False)
    efT_sizes = [2 * w for w in widths]
    ef_off = np.cumsum([0] + efT_sizes)
    efT = nc.declare_dram_parameter(
        "efT", [128, int(ef_off[-1])], BF16, isOutput=False
    )
    wT = nc.declare_dram_parameter("wT", [K, E], BF16, isOutput=False)
    bias = nc.declare_dram_parameter("bias", [E], F32, isOutput=False)
    # compact partition-major output: per (slot, gang) a [128, 4, W] block,
    # linear per partition. Host un-permutes + divides by row sums.
    pr_off = np.cumsum([0] + [8 * w for w in widths])  # per-slot elems/partition
    prob = nc.declare_dram_parameter(
        "probc", [128, int(pr_off[-1])], mybir.dt.uint8, isOutput=True
    )

    with tile.TileContext(nc) as tc, ExitStack() as ctx:
        singles = ctx.enter_context(tc.tile_pool(name="singles", bufs=1))
        pin = ctx.enter_context(tc.tile_pool(name="pin", bufs=4))
        pproj = ctx.enter_context(tc.tile_pool(name="pproj", bufs=3))
        pprob = ctx.enter_context(tc.tile_pool(name="pprob", bufs=3))
        pet = ctx.enter_context(tc.tile_pool(name="pet", bufs=4))
        psmall = ctx.enter_context(tc.tile_pool(name="psmall", bufs=16))
        pps1 = ctx.enter_context(tc.tile_pool(name="pps1", bufs=2, space="PSUM"))
        pps2 = ctx.enter_context(tc.tile_pool(name="pps2", bufs=3, space="PSUM"))

        # ---- resident constants (issued after the first uft half) ----
        wt_sb = singles.tile([128, 4, E], BF16)
        b_sb = singles.tile([128, 2], F32)

        def emit_consts():
            # consts ride the scalar HWDGE ring: they land in parallel with
            # uft0-h0 on the sync ring instead of queueing behind it
            # wT as 4 k-tiles: wt_sb[p, kt, e] = wT[kt*128+p, e]
            nc.scalar.dma_start(
                out=wt_sb, in_=wT[:, :].rearrange("(kt p) e -> p kt e", p=128)
            )
            # bias as 2 e-tiles on partitions: b_sb[p, et] = bias[et*128+p]
            nc.scalar.dma_start(
                out=b_sb, in_=bias[:].rearrange("(et p) -> p et", p=128)
            )

        def emit_uft_pair(bi):
            # one [128, 2, 4, U] tile holds batches (bi, bi+1): fewer, bigger
            # DMAs (16KB/partition) -> fewer ring slots + completion sems
            uftp = pin.tile([128, 2, 4, U], BF16, tag="uftp", name=f"uftp{bi}")
            if bi == 0:
                # ramp: u-half of batch 0 first so mm1 starts after 512KB
                nc.sync.dma_start(
                    out=uftp[:, 0, :, 0:512],
                    in_=ufT[0, :, :].rearrange("p (kt u) -> p kt u", kt=4)[
                        :, :, 0:512
                    ],
                )
                emit_consts()
                nc.sync.dma_start(
                    out=uftp[:, 0, :, 512:1024],
                    in_=ufT[0, :, :].rearrange("p (kt u) -> p kt u", kt=4)[
                        :, :, 512:1024
                    ],
                )
                if bpc > 1:
                    nc.sync.dma_start(
                        out=uftp[:, 1],
                        in_=ufT[1, :, :].rearrange("p (kt u) -> p kt u", kt=4),
                    )
            else:
                nc.sync.dma_start(
                    out=uftp,
                    in_=ufT[bi : bi + 2, :, :].rearrange(
                        "b p (kt u) -> p b kt u", kt=4
                    ),
                )
            return uftp

        def emit_eft(bi, eng=None):
            # the first two slots' efeat ride the (idle-at-start) scalar
            # ring so mm2 of slot 0 isn't gated behind 2MB of uft loads
            eng = eng or (nc.scalar if bi < 2 else nc.sync)
            W = widths[bi]
            eft = pin.tile([128, 2, W], BF16, tag="eft", name=f"eft{bi}")
            eng.dma_start(
                out=eft,
                in_=efT[:, int(ef_off[bi]) : int(ef_off[bi + 1])].rearrange(
                    "p (et n) -> p et n", et=2
                ),
            )
            return eft

        def emit_mm1_group(uft, projT, gi):
            # group gi -> (ej, uc), uc-major: both e-halves of u-chunk 0 come
            # first, so mm2 tiles u0..u3 unblock after 2 groups instead of 4
            ej, uc = gi % 2, gi // 2
            esl = slice(ej * 128, (ej + 1) * 128)
            usl = slice(uc * 512, (uc + 1) * 512)
            ps1 = pps1.tile([128, 512], F32, tag="ps1")
            for kj in range(4):
                nc.tensor.matmul(
                    ps1,
                    lhsT=wt_sb[:, kj, esl],
                    rhs=uft[:, kj, usl],
                    start=(kj == 0),
                    stop=(kj == 3),
                )
            # relu(x + b) = max(x + b, 0) fused on DVE; casts to bf16
            nc.vector.tensor_scalar(
                out=projT[:, ej, usl],
                in0=ps1,
                scalar1=b_sb[:, ej : ej + 1],
                scalar2=0.0,
                op0=mybir.AluOpType.add,
                op1=mybir.AluOpType.max,
            )

        pair_state = {}

        def emit_softmax_tile(bi, projT, eft, ui):
            # only the first widths[bi] columns are live
            W = widths[bi]
            nslices = [slice(0, min(512, W))] + ([slice(512, W)] if W > 512 else [])
            uslice = slice(ui * 128, (ui + 1) * 128)
            ps2 = pps2.tile([128, W], F32, tag="ps2", name=f"ps2_{bi}_{ui}")
            # e-major: consecutive matmuls share the same lhsT (weight reuse)
            for ej in range(2):
                for nsl in nslices:
                    nc.tensor.matmul(
                        ps2[:, nsl],
                        lhsT=projT[:, ej, uslice],
                        rhs=eft[:, ej, nsl],
                        start=(ej == 0),
                        stop=(ej == 1),
                    )
            # gang ALL 8 u-tiles of the slot into one [128, 8, W] uint8
            # store tile: one output DMA per slot (8 total)
            if ui == 0:
                pair_state["tile"] = pprob.tile(
                    [128, 8, W], mybir.dt.uint8, tag="prob", name=f"prob{bi}"
                )
            prob_t = pair_state["tile"]
            et = pet.tile([128, W], BF16, tag="et", name=f"et{bi}_{ui}")
            s = psmall.tile([128, 1], F32, tag="s")
            nc.scalar.activation(
                out=et,
                in_=ps2,
                func=mybir.ActivationFunctionType.Exp,
                scale=1.0 / 16.0,
                accum_out=s,
            )
            r = psmall.tile([128, 1], F32, tag="r")
            nc.vector.reciprocal(out=r, in_=s)
            # q = (et * r) * QSCALE -> uint8. QSCALE = 253.5 (not 255) so
            # bf16 jitter on a prob ~= 1.0 can never push q past 255 (numpy
            # wraps, HW saturates -- rely on neither). Conversion truncates
            # (or rounds, HW-dependent): either way abs error <= 1 LSB =
            # 3.9e-3, under the ~1.2e-2 absolute budget, and the output
            # stream halves vs bf16. Host divides by QSCALE.
            nc.vector.tensor_scalar(
                out=prob_t[:, ui, :],
                in0=et,
                scalar1=r,
                scalar2=253.5,
                op0=mybir.AluOpType.mult,
                op1=mybir.AluOpType.mult,
            )
            if ui == 7:
                off = int(pr_off[bi])
                # stores alternate between the two HWDGE rings (SP / ACT):
                # the store direction pays HBM-write receipts and runs at
                # ~half line rate per ring, so two rings drain in parallel
                eng = nc.sync if bi % 2 == 0 else nc.scalar
                eng.dma_start(
                    out=prob[:, off : off + 8 * W].rearrange(
                        "p (j n) -> p j n", j=8
                    ),
                    in_=prob_t,
                )

        # Software-pipelined emission: mm1 groups for batch bi+1 are emitted
        # between softmax tiles of batch bi's second half, so the PE never
        # monopolizes a contiguous ~4us window on mm1 while ACT's 3-deep
        # PSUM backlog drains.
        pairs = {0: emit_uft_pair(0)}
        efts = {bi: emit_eft(bi) for bi in range(min(3, bpc))}
        projs = {0: pproj.tile([128, 2, U], BF16, tag="projT", name="projT0")}

        def uft_view(bi):
            return pairs[bi - bi % 2][:, bi % 2]

        for gi in range(4):
            emit_mm1_group(uft_view(0), projs[0], gi)
        for bi in range(bpc):
            eft = efts[bi]
            projT = projs[bi]
            nxt = bi + 1
            if nxt < bpc and nxt % 2 == 0:
                pairs[nxt] = emit_uft_pair(nxt)
            if nxt + 2 < bpc:
                efts[nxt + 2] = emit_eft(nxt + 2)
            for ui in range(4):
                emit_softmax_tile(bi, projT, eft, ui)
            if nxt < bpc:
                projs[nxt] = pproj.tile(
                    [128, 2, U], BF16, tag="projT", name=f"projT{nxt}"
                )
            for ui in range(4, 8):
                emit_softmax_tile(bi, projT, eft, ui)
                if nxt < bpc:
                    emit_mm1_group(uft_view(nxt), projs[nxt], ui - 4)

    # Runs Bacc.compile(): register allocation + event-semaphore splitting.
    nc.finalize()
    return nc


def _widths_for(num_enemy):
    ne = np.asarray(num_enemy).astype(np.int64)
    ne_eff = np.where(ne > 0, ne, N)
    order = np.argsort(-ne_eff, kind="stable")
    slot_ne = ne_eff[order].reshape(BPC, N_CORES)
    widths = tuple(
        int(max(128, -(-int(m) // 64) * 64)) for m in slot_ne.max(axis=1)
    )
    return order, widths


def _prep_inputs(ufeat, efeat, num_enemy, v, g, b, widths):
    """Host prep: weight-norm, bf16 cast, mask poison, SBUF-layout staging."""
    ufeat = np.asarray(ufeat, dtype=np.float32)
    efeat = np.asarray(efeat, dtype=np.float32)
    num_enemy = np.asarray(num_enemy).astype(np.int64)
    v = np.asarray(v, dtype=np.float32)
    g = np.float32(np.asarray(g))
    b = np.asarray(b, dtype=np.float32)

    W = (g / np.float32(np.linalg.norm(v))) * v  # [E, K]
    wT = np.ascontiguousarray(W.T).astype(BF16_NP)  # [K, E]

    # SBUF layout: ufT[b, p, kt*U + u] = ufeat[b, u, kt*128+p]
    # [B, U, K] -> bf16 -> [B, K, U] -> [B, 4, 128, U] -> [B, 128, 4, U]
    ufT = (
        ufeat.astype(BF16_NP)
        .transpose(0, 2, 1)
        .reshape(B, 4, 128, U)
        .transpose(0, 2, 1, 3)
        .reshape(B, 128, 4 * U)
    )

    # efT in SBUF layout [B, 128, 2, N]: efT[b, p, et, n] = efeat[b, n, et*128+p]
    efT = efeat.astype(BF16_NP).transpose(0, 2, 1)  # [B, E, N]
    # Mask: poison masked efeat columns (n >= num_enemy) with -1e30. Since
    # proj >= 0 (relu) and a proj row is never identically 0 in practice,
    # masked logits land at <= -1e28 and exp underflows to exactly 0 — the
    # same 0 the reference's -1e9 bias produces. num_enemy==0 => all lanes
    # masked => uniform shift cancels in softmax => leave unpoisoned.
    ne = np.where(num_enemy > 0, num_enemy, N)
    col_masked = np.arange(N)[None, :] >= ne[:, None]  # [B, N]
    efT[np.broadcast_to(col_masked[:, None, :], efT.shape)] = BF16_NP(-1e30)
    efT = efT.reshape(B, 2, 128, N).transpose(0, 2, 1, 3)  # [B, 128, 2, N]

    return ufT, efT, wT, b


def _pack_ef(efT, perm, widths):
    """Per-core packed efeat: [128, sum(2*W)] per the compiled offsets."""
    cols = []
    for k, bi in enumerate(perm):
        w = widths[k]
        cols.append(efT[bi, :, :, :w].reshape(128, 2 * w))
    return np.ascontiguousarray(np.concatenate(cols, axis=1))


def _unpack_out(probc, widths):
    """[128, sum(8*W)] uint8 -> [bpc, U, N] f32 (q/255)."""
    bpc = len(widths)
    out = np.zeros((bpc, U, N), dtype=np.float32)
    off = 0
    for k, w in enumerate(widths):
        blk = probc[:, off : off + 8 * w].reshape(128, 8, w)
        # u = j*128 + p
        rows = blk.transpose(1, 0, 2).reshape(U, w).astype(np.float32)
        out[k, :, :w] = rows * np.float32(1.0 / 253.5)
        off += 8 * w
    return out


_nc_cache: dict[tuple, bass.Bass] = {}


def run(ufeat, efeat, num_enemy, v, g, b, trace=False):
    order, widths = _widths_for(num_enemy)
    ufT, efT, wT, b = _prep_inputs(ufeat, efeat, num_enemy, v, g, b, widths)

    key = (BPC, widths)
    if key not in _nc_cache:
        _nc_cache[key] = _build_bass(BPC, widths)
    nc = _nc_cache[key]

    in_maps = []
    perms = []
    for c in range(N_CORES):
        perm = order.reshape(BPC, N_CORES)[:, c]  # batch index for each slot
        perms.append(perm)
        in_maps.append(
            {
                "ufT": np.ascontiguousarray(ufT[perm]),
                "efT": _pack_ef(efT, perm, widths),
                "wT": wT,
                "bias": b,
            }
        )

    res = run_bass_kernel_spmd(nc, in_maps, list(range(N_CORES)), trace=trace)
    out = np.empty((B, U, N), dtype=np.float32)
    for c in range(N_CORES):
        probc = np.asarray(res.results[c]["probc"])
        out[perms[c]] = _unpack_out(probc, widths)
    return out, res


def kernel(ufeat, efeat, num_enemy, v, g, b):
    out, _ = run(ufeat, efeat, num_enemy, v, g, b, trace=False)
    return out


# revision 16
# speedup vs baseline: 1.0278x; 1.0278x over previous
"""DotAttackHead kernel for Trainium2 (8 NeuronCores, data-parallel over batch).

prob = softmax(relu(ufeat @ W.T + b) @ efeat.T / sqrt(256) + mask_bias)
W = g * v / ||v||_F

Sharding: batch 64 -> 8 cores x 8 batches (data-parallel). Params replicated.

Host prep: weight-norm W, bf16 cast, mask folded into efeat (masked columns
poisoned with -1e30 so masked logits underflow exp to exactly 0), and BOTH
inputs staged in the exact SBUF tile layout ([p, kt, u] / [p, et, n]) so
every load is a pure linear 128-partition DMA with 4-8KB contiguous
per-partition chunks (the naive [K, U]-transposing load produces 1KB
descriptors and runs at ~1/4 line rate).

Device per batch b (software-pipelined across batches):
  mm1:  projT[e,u] = relu(wT.T @ ufT[b] + bias)   (PE bf16; bias+relu fused
        on DVE as tensor_scalar add/max reading PSUM, bf16 out)
  mm2:  psum[u,n]  = projT.T @ efT[b]             (PE bf16, fp32 PSUM)
  soft: et = Exp(psum/16) with accum_out row-sum (ACT, bf16, written
        directly into the ganged [128,4,W] store tile). The softmax
        division happens on the HOST during the f32 upcast (prob = et / s),
        which removes the reciprocal+multiply (35us of DVE) from the device.
Row sums gang into one [128, bpc, 8] tile, DMA'd once at the end.

Output is stored in a COMPACT partition-major layout: one flat bf16 tensor,
per (slot, 4-u-tile gang) a [128, 4, W] block written as one linear DMA
(4W*2B contiguous per partition -> max-size descriptors). The host
un-permutes to [U, N], divides by the row sums and zero-fills columns
[W, N). Masked-width specialization: batches sorted by effective width
descending, rank 8k+c -> (core c, slot k), slot width = slot max rounded
up to 128, compiled per-widths (NEFF-cached).

No max-subtraction: logits are O(+-6) so exp is safe in fp32, and softmax is
shift-invariant, so this matches the reference.

fp8 was evaluated for the matmuls (DoubleRow, 2x PE) and REJECTED: e4m3
quantization of proj+efeat pushes rel err to 4e-2 > the 2e-2 gate.
"""

from contextlib import ExitStack

import ml_dtypes
import numpy as np

import concourse.bass as bass
import concourse.mybir as mybir
import concourse.tile as tile
from concourse import bacc
from concourse.bass_utils import run_bass_kernel_spmd

N_CORES = 8
B = 64
U = 1024  # units
E = 256   # efeat dim
K = 512   # ufeat dim
N = 1024  # enemies
BPC = B // N_CORES  # batches per core

F32 = mybir.dt.float32
BF16 = mybir.dt.bfloat16
BF16_NP = ml_dtypes.bfloat16

def _build_bass(bpc: int = BPC, widths: tuple = ()) -> bass.Bass:
    if not widths:
        widths = (N,) * bpc
    assert len(widths) == bpc and all(w % 64 == 0 and 128 <= w <= N for w in widths)
    # Bacc (not raw Bass): its finalize() runs generate_event_semaphores,
    # which splits multi-wait instructions to satisfy TRN2's 1-wait limit.
    nc = bacc.Bacc(None, target_bir_lowering=False)

    # inputs staged host-side in SBUF layout: linear 128-partition loads
    ufT = nc.declare_dram_parameter("ufT", [bpc, 128, 4 * U], BF16, isOutput=False)
    efT_sizes = [2 * w for w in widths]
    ef_off = np.cumsum([0] + efT_sizes)
    efT = nc.declare_dram_parameter(
        "efT", [128, int(ef_off[-1])], BF16, isOutput=False
    )
    wT = nc.declare_dram_parameter("wT", [K, E], BF16, isOutput=False)
    bias = nc.declare_dram_parameter("bias", [E], F32, isOutput=False)
    # compact partition-major output: per (slot, gang) a [128, 4, W] block,
    # linear per partition. Host un-permutes + divides by row sums.
    pr_off = np.cumsum([0] + [8 * w for w in widths])  # per-slot elems/partition
    prob = nc.declare_dram_parameter(
        "probc", [128, int(pr_off[-1])], mybir.dt.uint8, isOutput=True
    )

    with tile.TileContext(nc) as tc, ExitStack() as ctx:
        singles = ctx.enter_context(tc.tile_pool(name="singles", bufs=1))
        pin = ctx.enter_context(tc.tile_pool(name="pin", bufs=4))
        pproj = ctx.enter_context(tc.tile_pool(name="pproj", bufs=3))
        pprob = ctx.enter_context(tc.tile_pool(name="pprob", bufs=3))
        pet = ctx.enter_context(tc.tile_pool(name="pet", bufs=4))
        psmall = ctx.enter_context(tc.tile_pool(name="psmall", bufs=16))
        pps1 = ctx.enter_context(tc.tile_pool(name="pps1", bufs=2, space="PSUM"))
        pps2 = ctx.enter_context(tc.tile_pool(name="pps2", bufs=3, space="PSUM"))

        # ---- resident constants (issued after the first uft half) ----
        wt_sb = singles.tile([128, 4, E], BF16)
        b_sb = singles.tile([128, 2], F32)

        def emit_consts():
            # consts ride the scalar HWDGE ring: they land in parallel with
            # uft0-h0 on the sync ring instead of queueing behind it
            # wT as 4 k-tiles: wt_sb[p, kt, e] = wT[kt*128+p, e]
            nc.scalar.dma_start(
                out=wt_sb, in_=wT[:, :].rearrange("(kt p) e -> p kt e", p=128)
            )
            # bias as 2 e-tiles on partitions: b_sb[p, et] = bias[et*128+p]
            nc.scalar.dma_start(
                out=b_sb, in_=bias[:].rearrange("(et p) -> p et", p=128)
            )

        def emit_uft_pair(bi):
            # one [128, 2, 4, U] tile holds batches (bi, bi+1): fewer, bigger
            # DMAs (16KB/partition) -> fewer ring slots + completion sems
            uftp = pin.tile([128, 2, 4, U], BF16, tag="uftp", name=f"uftp{bi}")
            if bi == 0:
                # ramp: kt0 of u-half 0 first (128KB) so the very first
                # matmul's DMA-completion wait resolves ~2us sooner; then
                # the rest of half 0, then half 1
                nc.sync.dma_start(
                    out=uftp[:, 0, 0:1, 0:512],
                    in_=ufT[0, :, :].rearrange("p (kt u) -> p kt u", kt=4)[
                        :, 0:1, 0:512
                    ],
                )
                emit_consts()
                nc.sync.dma_start(
                    out=uftp[:, 0, 1:4, 0:512],
                    in_=ufT[0, :, :].rearrange("p (kt u) -> p kt u", kt=4)[
                        :, 1:4, 0:512
                    ],
                )
                nc.sync.dma_start(
                    out=uftp[:, 0, :, 512:1024],
                    in_=ufT[0, :, :].rearrange("p (kt u) -> p kt u", kt=4)[
                        :, :, 512:1024
                    ],
                )
                if bpc > 1:
                    nc.sync.dma_start(
                        out=uftp[:, 1],
                        in_=ufT[1, :, :].rearrange("p (kt u) -> p kt u", kt=4),
                    )
            else:
                nc.sync.dma_start(
                    out=uftp,
                    in_=ufT[bi : bi + 2, :, :].rearrange(
                        "b p (kt u) -> p b kt u", kt=4
                    ),
                )
            return uftp

        def emit_eft(bi, eng=None):
            # the first two slots' efeat ride the (idle-at-start) scalar
            # ring so mm2 of slot 0 isn't gated behind 2MB of uft loads
            eng = eng or (nc.scalar if bi < 2 else nc.sync)
            W = widths[bi]
            eft = pin.tile([128, 2, W], BF16, tag="eft", name=f"eft{bi}")
            eng.dma_start(
                out=eft,
                in_=efT[:, int(ef_off[bi]) : int(ef_off[bi + 1])].rearrange(
                    "p (et n) -> p et n", et=2
                ),
            )
            return eft

        def emit_mm1_group(uft, projT, gi):
            # group gi -> (ej, uc), uc-major: both e-halves of u-chunk 0 come
            # first, so mm2 tiles u0..u3 unblock after 2 groups instead of 4
            ej, uc = gi % 2, gi // 2
            esl = slice(ej * 128, (ej + 1) * 128)
            usl = slice(uc * 512, (uc + 1) * 512)
            ps1 = pps1.tile([128, 512], F32, tag="ps1")
            for kj in range(4):
                nc.tensor.matmul(
                    ps1,
                    lhsT=wt_sb[:, kj, esl],
                    rhs=uft[:, kj, usl],
                    start=(kj == 0),
                    stop=(kj == 3),
                )
            # relu(x + b) = max(x + b, 0) fused on DVE; casts to bf16
            nc.vector.tensor_scalar(
                out=projT[:, ej, usl],
                in0=ps1,
                scalar1=b_sb[:, ej : ej + 1],
                scalar2=0.0,
                op0=mybir.AluOpType.add,
                op1=mybir.AluOpType.max,
            )

        pair_state = {}

        def emit_softmax_tile(bi, projT, eft, ui):
            # only the first widths[bi] columns are live
            W = widths[bi]
            nslices = [slice(0, min(512, W))] + ([slice(512, W)] if W > 512 else [])
            uslice = slice(ui * 128, (ui + 1) * 128)
            ps2 = pps2.tile([128, W], F32, tag="ps2", name=f"ps2_{bi}_{ui}")
            # e-major: consecutive matmuls share the same lhsT (weight reuse)
            for ej in range(2):
                for nsl in nslices:
                    nc.tensor.matmul(
                        ps2[:, nsl],
                        lhsT=projT[:, ej, uslice],
                        rhs=eft[:, ej, nsl],
                        start=(ej == 0),
                        stop=(ej == 1),
                    )
            # gang ALL 8 u-tiles of the slot into one [128, 8, W] uint8
            # store tile: one output DMA per slot (8 total)
            if ui == 0:
                pair_state["tile"] = pprob.tile(
                    [128, 8, W], mybir.dt.uint8, tag="prob", name=f"prob{bi}"
                )
            prob_t = pair_state["tile"]
            et = pet.tile([128, W], BF16, tag="et", name=f"et{bi}_{ui}")
            s = psmall.tile([128, 1], F32, tag="s")
            nc.scalar.activation(
                out=et,
                in_=ps2,
                func=mybir.ActivationFunctionType.Exp,
                scale=1.0 / 16.0,
                accum_out=s,
            )
            r = psmall.tile([128, 1], F32, tag="r")
            nc.vector.reciprocal(out=r, in_=s)
            # q = (et * r) * QSCALE -> uint8. QSCALE = 253.5 (not 255) so
            # bf16 jitter on a prob ~= 1.0 can never push q past 255 (numpy
            # wraps, HW saturates -- rely on neither). Conversion truncates
            # (or rounds, HW-dependent): either way abs error <= 1 LSB =
            # 3.9e-3, under the ~1.2e-2 absolute budget, and the output
            # stream halves vs bf16. Host divides by QSCALE.
            nc.vector.tensor_scalar(
                out=prob_t[:, ui, :],
                in0=et,
                scalar1=r,
                scalar2=253.5,
                op0=mybir.AluOpType.mult,
                op1=mybir.AluOpType.mult,
            )
            if ui == 7:
                off = int(pr_off[bi])
                # stores alternate between the two HWDGE rings (SP / ACT):
                # the store direction pays HBM-write receipts and runs at
                # ~half line rate per ring, so two rings drain in parallel
                eng = nc.sync if bi % 2 == 0 else nc.scalar
                eng.dma_start(
                    out=prob[:, off : off + 8 * W].rearrange(
                        "p (j n) -> p j n", j=8
                    ),
                    in_=prob_t,
                )

        # Software-pipelined emission: mm1 groups for batch bi+1 are emitted
        # between softmax tiles of batch bi's second half, so the PE never
        # monopolizes a contiguous ~4us window on mm1 while ACT's 3-deep
        # PSUM backlog drains.
        pairs = {0: emit_uft_pair(0)}
        efts = {bi: emit_eft(bi) for bi in range(min(3, bpc))}
        projs = {0: pproj.tile([128, 2, U], BF16, tag="projT", name="projT0")}

        def uft_view(bi):
            return pairs[bi - bi % 2][:, bi % 2]

        for gi in range(4):
            emit_mm1_group(uft_view(0), projs[0], gi)
        for bi in range(bpc):
            eft = efts[bi]
            projT = projs[bi]
            nxt = bi + 1
            if nxt < bpc and nxt % 2 == 0:
                pairs[nxt] = emit_uft_pair(nxt)
            if nxt + 2 < bpc:
                efts[nxt + 2] = emit_eft(nxt + 2)
            for ui in range(4):
                emit_softmax_tile(bi, projT, eft, ui)
            if nxt < bpc:
                projs[nxt] = pproj.tile(
                    [128, 2, U], BF16, tag="projT", name=f"projT{nxt}"
                )
            for ui in range(4, 8):
                emit_softmax_tile(bi, projT, eft, ui)
                if nxt < bpc:
                    emit_mm1_group(uft_view(nxt), projs[nxt], ui - 4)

    # Runs Bacc.compile(): register allocation + event-semaphore splitting.
    nc.finalize()
    return nc


def _widths_for(num_enemy):
    ne = np.asarray(num_enemy).astype(np.int64)
    ne_eff = np.where(ne > 0, ne, N)
    order = np.argsort(-ne_eff, kind="stable")
    slot_ne = ne_eff[order].reshape(BPC, N_CORES)
    widths = tuple(
        int(max(128, -(-int(m) // 64) * 64)) for m in slot_ne.max(axis=1)
    )
    return order, widths


def _prep_inputs(ufeat, efeat, num_enemy, v, g, b, widths):
    """Host prep: weight-norm, bf16 cast, mask poison, SBUF-layout staging."""
    ufeat = np.asarray(ufeat, dtype=np.float32)
    efeat = np.asarray(efeat, dtype=np.float32)
    num_enemy = np.asarray(num_enemy).astype(np.int64)
    v = np.asarray(v, dtype=np.float32)
    g = np.float32(np.asarray(g))
    b = np.asarray(b, dtype=np.float32)

    W = (g / np.float32(np.linalg.norm(v))) * v  # [E, K]
    wT = np.ascontiguousarray(W.T).astype(BF16_NP)  # [K, E]

    # SBUF layout: ufT[b, p, kt*U + u] = ufeat[b, u, kt*128+p]
    # [B, U, K] -> bf16 -> [B, K, U] -> [B, 4, 128, U] -> [B, 128, 4, U]
    ufT = (
        ufeat.astype(BF16_NP)
        .transpose(0, 2, 1)
        .reshape(B, 4, 128, U)
        .transpose(0, 2, 1, 3)
        .reshape(B, 128, 4 * U)
    )

    # efT in SBUF layout [B, 128, 2, N]: efT[b, p, et, n] = efeat[b, n, et*128+p]
    efT = efeat.astype(BF16_NP).transpose(0, 2, 1)  # [B, E, N]
    # Mask: poison masked efeat columns (n >= num_enemy) with -1e30. Since
    # proj >= 0 (relu) and a proj row is never identically 0 in practice,
    # masked logits land at <= -1e28 and exp underflows to exactly 0 — the
    # same 0 the reference's -1e9 bias produces. num_enemy==0 => all lanes
    # masked => uniform shift cancels in softmax => leave unpoisoned.
    ne = np.where(num_enemy > 0, num_enemy, N)
    col_masked = np.arange(N)[None, :] >= ne[:, None]  # [B, N]
    efT[np.broadcast_to(col_masked[:, None, :], efT.shape)] = BF16_NP(-1e30)
    efT = efT.reshape(B, 2, 128, N).transpose(0, 2, 1, 3)  # [B, 128, 2, N]

    return ufT, efT, wT, b


def _pack_ef(efT, perm, widths):
    """Per-core packed efeat: [128, sum(2*W)] per the compiled offsets."""
    cols = []
    for k, bi in enumerate(perm):
        w = widths[k]
        cols.append(efT[bi, :, :, :w].reshape(128, 2 * w))
    return np.ascontiguousarray(np.concatenate(cols, axis=1))


def _unpack_out(probc, widths):
    """[128, sum(8*W)] uint8 -> [bpc, U, N] f32 (q/255)."""
    bpc = len(widths)
    out = np.zeros((bpc, U, N), dtype=np.float32)
    off = 0
    for k, w in enumerate(widths):
        blk = probc[:, off : off + 8 * w].reshape(128, 8, w)
        # u = j*128 + p
        rows = blk.transpose(1, 0, 2).reshape(U, w).astype(np.float32)
        out[k, :, :w] = rows * np.float32(1.0 / 253.5)
        off += 8 * w
    return out


_nc_cache: dict[tuple, bass.Bass] = {}


def run(ufeat, efeat, num_enemy, v, g, b, trace=False):
    order, widths = _widths_for(num_enemy)
    ufT, efT, wT, b = _prep_inputs(ufeat, efeat, num_enemy, v, g, b, widths)

    key = (BPC, widths)
    if key not in _nc_cache:
        _nc_cache[key] = _build_bass(BPC, widths)
    nc = _nc_cache[key]

    in_maps = []
    perms = []
    for c in range(N_CORES):
        perm = order.reshape(BPC, N_CORES)[:, c]  # batch index for each slot
        perms.append(perm)
        in_maps.append(
            {
                "ufT": np.ascontiguousarray(ufT[perm]),
                "efT": _pack_ef(efT, perm, widths),
                "wT": wT,
                "bias": b,
            }
        )

    res = run_bass_kernel_spmd(nc, in_maps, list(range(N_CORES)), trace=trace)
    out = np.empty((B, U, N), dtype=np.float32)
    for c in range(N_CORES):
        probc = np.asarray(res.results[c]["probc"])
        out[perms[c]] = _unpack_out(probc, widths)
    return out, res


def kernel(ufeat, efeat, num_enemy, v, g, b):
    out, _ = run(ufeat, efeat, num_enemy, v, g, b, trace=False)
    return out


# revision 17
# speedup vs baseline: 1.0334x; 1.0054x over previous
"""DotAttackHead kernel for Trainium2 (8 NeuronCores, data-parallel over batch).

prob = softmax(relu(ufeat @ W.T + b) @ efeat.T / sqrt(256) + mask_bias)
W = g * v / ||v||_F

Sharding: batch 64 -> 8 cores x 8 batches (data-parallel). Params replicated.

Host prep: weight-norm W, bf16 cast, mask folded into efeat (masked columns
poisoned with -1e30 so masked logits underflow exp to exactly 0), and BOTH
inputs staged in the exact SBUF tile layout ([p, kt, u] / [p, et, n]) so
every load is a pure linear 128-partition DMA with 4-8KB contiguous
per-partition chunks (the naive [K, U]-transposing load produces 1KB
descriptors and runs at ~1/4 line rate).

Device per batch b (software-pipelined across batches):
  mm1:  projT[e,u] = relu(wT.T @ ufT[b] + bias)   (PE bf16; bias+relu fused
        on DVE as tensor_scalar add/max reading PSUM, bf16 out)
  mm2:  psum[u,n]  = projT.T @ efT[b]             (PE bf16, fp32 PSUM)
  soft: et = Exp(psum/16) with accum_out row-sum (ACT, bf16, written
        directly into the ganged [128,4,W] store tile). The softmax
        division happens on the HOST during the f32 upcast (prob = et / s),
        which removes the reciprocal+multiply (35us of DVE) from the device.
Row sums gang into one [128, bpc, 8] tile, DMA'd once at the end.

Output is stored in a COMPACT partition-major layout: one flat bf16 tensor,
per (slot, 4-u-tile gang) a [128, 4, W] block written as one linear DMA
(4W*2B contiguous per partition -> max-size descriptors). The host
un-permutes to [U, N], divides by the row sums and zero-fills columns
[W, N). Masked-width specialization: batches sorted by effective width
descending, rank 8k+c -> (core c, slot k), slot width = slot max rounded
up to 128, compiled per-widths (NEFF-cached).

No max-subtraction: logits are O(+-6) so exp is safe in fp32, and softmax is
shift-invariant, so this matches the reference.

fp8 was evaluated for the matmuls (DoubleRow, 2x PE) and REJECTED: e4m3
quantization of proj+efeat pushes rel err to 4e-2 > the 2e-2 gate.
"""

from contextlib import ExitStack

import ml_dtypes
import numpy as np

import concourse.bass as bass
import concourse.mybir as mybir
import concourse.tile as tile
from concourse import bacc
from concourse.bass_utils import run_bass_kernel_spmd

N_CORES = 8
B = 64
U = 1024  # units
E = 256   # efeat dim
K = 512   # ufeat dim
N = 1024  # enemies
BPC = B // N_CORES  # batches per core

F32 = mybir.dt.float32
BF16 = mybir.dt.bfloat16
BF16_NP = ml_dtypes.bfloat16

def _build_bass(bpc: int = BPC, widths: tuple = ()) -> bass.Bass:
    if not widths:
        widths = (N,) * bpc
    assert len(widths) == bpc and all(w % 64 == 0 and 128 <= w <= N for w in widths)
    # Bacc (not raw Bass): its finalize() runs generate_event_semaphores,
    # which splits multi-wait instructions to satisfy TRN2's 1-wait limit.
    nc = bacc.Bacc(None, target_bir_lowering=False)

    # inputs staged host-side in SBUF layout: linear 128-partition loads
    ufT = nc.declare_dram_parameter("ufT", [bpc, 128, 4 * U], BF16, isOutput=False)
    efT_sizes = [2 * w for w in widths]
    ef_off = np.cumsum([0] + efT_sizes)
    efT = nc.declare_dram_parameter(
        "efT", [128, int(ef_off[-1])], BF16, isOutput=False
    )
    wT = nc.declare_dram_parameter("wT", [K, E], BF16, isOutput=False)
    bias = nc.declare_dram_parameter("bias", [E], F32, isOutput=False)
    # compact partition-major output: per (slot, gang) a [128, 4, W] block,
    # linear per partition. Host un-permutes + divides by row sums.
    pr_off = np.cumsum([0] + [8 * w for w in widths])  # per-slot elems/partition
    prob = nc.declare_dram_parameter(
        "probc", [128, int(pr_off[-1])], mybir.dt.uint8, isOutput=True
    )

    with tile.TileContext(nc) as tc, ExitStack() as ctx:
        singles = ctx.enter_context(tc.tile_pool(name="singles", bufs=1))
        pin = ctx.enter_context(tc.tile_pool(name="pin", bufs=4))
        pproj = ctx.enter_context(tc.tile_pool(name="pproj", bufs=3))
        pprob = ctx.enter_context(tc.tile_pool(name="pprob", bufs=3))
        pet = ctx.enter_context(tc.tile_pool(name="pet", bufs=4))
        psmall = ctx.enter_context(tc.tile_pool(name="psmall", bufs=16))
        pps1 = ctx.enter_context(tc.tile_pool(name="pps1", bufs=2, space="PSUM"))
        pps2 = ctx.enter_context(tc.tile_pool(name="pps2", bufs=3, space="PSUM"))

        # ---- resident constants (issued after the first uft half) ----
        wt_sb = singles.tile([128, 4, E], BF16)
        b_sb = singles.tile([128, 2], F32)

        def emit_consts():
            # consts ride the scalar HWDGE ring: they land in parallel with
            # uft0-h0 on the sync ring instead of queueing behind it
            # wT as 4 k-tiles: wt_sb[p, kt, e] = wT[kt*128+p, e]
            nc.scalar.dma_start(
                out=wt_sb, in_=wT[:, :].rearrange("(kt p) e -> p kt e", p=128)
            )
            # bias as 2 e-tiles on partitions: b_sb[p, et] = bias[et*128+p]
            nc.scalar.dma_start(
                out=b_sb, in_=bias[:].rearrange("(et p) -> p et", p=128)
            )

        def emit_uft_pair(bi):
            # one [128, 2, 4, U] tile holds batches (bi, bi+1): fewer, bigger
            # DMAs (16KB/partition) -> fewer ring slots + completion sems
            uftp = pin.tile([128, 2, 4, U], BF16, tag="uftp", name=f"uftp{bi}")
            if bi == 0:
                # ramp: kt0 of u-half 0 first (128KB) so the very first
                # matmul's DMA-completion wait resolves ~2us sooner; then
                # the rest of half 0, then half 1
                nc.sync.dma_start(
                    out=uftp[:, 0, 0:1, 0:512],
                    in_=ufT[0, :, :].rearrange("p (kt u) -> p kt u", kt=4)[
                        :, 0:1, 0:512
                    ],
                )
                emit_consts()
                nc.sync.dma_start(
                    out=uftp[:, 0, 1:4, 0:512],
                    in_=ufT[0, :, :].rearrange("p (kt u) -> p kt u", kt=4)[
                        :, 1:4, 0:512
                    ],
                )
                nc.sync.dma_start(
                    out=uftp[:, 0, :, 512:1024],
                    in_=ufT[0, :, :].rearrange("p (kt u) -> p kt u", kt=4)[
                        :, :, 512:1024
                    ],
                )
                if bpc > 1:
                    nc.sync.dma_start(
                        out=uftp[:, 1],
                        in_=ufT[1, :, :].rearrange("p (kt u) -> p kt u", kt=4),
                    )
            else:
                nc.sync.dma_start(
                    out=uftp,
                    in_=ufT[bi : bi + 2, :, :].rearrange(
                        "b p (kt u) -> p b kt u", kt=4
                    ),
                )
            return uftp

        def emit_eft(bi, eng=None):
            # the first two slots' efeat ride the (idle-at-start) scalar
            # ring so mm2 of slot 0 isn't gated behind 2MB of uft loads
            eng = eng or (nc.scalar if bi < 2 else nc.sync)
            W = widths[bi]
            eft = pin.tile([128, 2, W], BF16, tag="eft", name=f"eft{bi}")
            eng.dma_start(
                out=eft,
                in_=efT[:, int(ef_off[bi]) : int(ef_off[bi + 1])].rearrange(
                    "p (et n) -> p et n", et=2
                ),
            )
            return eft

        def emit_mm1_group(uft, projT, gi):
            # group gi -> (ej, uc), uc-major: both e-halves of u-chunk 0 come
            # first, so mm2 tiles u0..u3 unblock after 2 groups instead of 4
            ej, uc = gi % 2, gi // 2
            esl = slice(ej * 128, (ej + 1) * 128)
            usl = slice(uc * 512, (uc + 1) * 512)
            ps1 = pps1.tile([128, 512], F32, tag="ps1")
            for kj in range(4):
                nc.tensor.matmul(
                    ps1,
                    lhsT=wt_sb[:, kj, esl],
                    rhs=uft[:, kj, usl],
                    start=(kj == 0),
                    stop=(kj == 3),
                )
            # relu(x + b) = max(x + b, 0) fused on DVE; casts to bf16
            nc.vector.tensor_scalar(
                out=projT[:, ej, usl],
                in0=ps1,
                scalar1=b_sb[:, ej : ej + 1],
                scalar2=0.0,
                op0=mybir.AluOpType.add,
                op1=mybir.AluOpType.max,
            )

        pair_state = {}

        def emit_softmax_tile(bi, projT, eft, ui):
            # only the first widths[bi] columns are live
            W = widths[bi]
            nslices = [slice(0, min(512, W))] + ([slice(512, W)] if W > 512 else [])
            uslice = slice(ui * 128, (ui + 1) * 128)
            ps2 = pps2.tile([128, W], F32, tag="ps2", name=f"ps2_{bi}_{ui}")
            # e-major: consecutive matmuls share the same lhsT (weight reuse)
            for ej in range(2):
                for nsl in nslices:
                    nc.tensor.matmul(
                        ps2[:, nsl],
                        lhsT=projT[:, ej, uslice],
                        rhs=eft[:, ej, nsl],
                        start=(ej == 0),
                        stop=(ej == 1),
                    )
            # gang ALL 8 u-tiles of the slot into one [128, 8, W] uint8
            # store tile: one output DMA per slot (8 total)
            if ui == 0:
                pair_state["tile"] = pprob.tile(
                    [128, 8, W], mybir.dt.uint8, tag="prob", name=f"prob{bi}"
                )
            prob_t = pair_state["tile"]
            et = pet.tile([128, W], BF16, tag="et", name=f"et{bi}_{ui}")
            s = psmall.tile([128, 1], F32, tag="s")
            nc.scalar.activation(
                out=et,
                in_=ps2,
                func=mybir.ActivationFunctionType.Exp,
                scale=1.0 / 16.0,
                accum_out=s,
            )
            r = psmall.tile([128, 1], F32, tag="r")
            nc.vector.reciprocal(out=r, in_=s)
            # q = (et * r) * QSCALE -> uint8. QSCALE = 253.5 (not 255) so
            # bf16 jitter on a prob ~= 1.0 can never push q past 255 (numpy
            # wraps, HW saturates -- rely on neither). Conversion truncates
            # (or rounds, HW-dependent): either way abs error <= 1 LSB =
            # 3.9e-3, under the ~1.2e-2 absolute budget, and the output
            # stream halves vs bf16. Host divides by QSCALE.
            nc.vector.tensor_scalar(
                out=prob_t[:, ui, :],
                in0=et,
                scalar1=r,
                scalar2=253.5,
                op0=mybir.AluOpType.mult,
                op1=mybir.AluOpType.mult,
            )
            if ui == 7:
                off = int(pr_off[bi])
                # stores alternate between the two HWDGE rings (SP / ACT):
                # the store direction pays HBM-write receipts and runs at
                # ~half line rate per ring, so two rings drain in parallel
                eng = nc.sync if bi % 2 == 0 else nc.scalar
                eng.dma_start(
                    out=prob[:, off : off + 8 * W].rearrange(
                        "p (j n) -> p j n", j=8
                    ),
                    in_=prob_t,
                )

        # Software-pipelined emission: mm1 groups for batch bi+1 are emitted
        # between softmax tiles of batch bi's second half, so the PE never
        # monopolizes a contiguous ~4us window on mm1 while ACT's 3-deep
        # PSUM backlog drains.
        pairs = {0: emit_uft_pair(0)}
        efts = {bi: emit_eft(bi) for bi in range(min(3, bpc))}
        projs = {0: pproj.tile([128, 2, U], BF16, tag="projT", name="projT0")}

        def uft_view(bi):
            return pairs[bi - bi % 2][:, bi % 2]

        for gi in range(4):
            emit_mm1_group(uft_view(0), projs[0], gi)
        for bi in range(bpc):
            eft = efts[bi]
            projT = projs[bi]
            nxt = bi + 1
            if nxt < bpc and nxt % 2 == 0:
                pairs[nxt] = emit_uft_pair(nxt)
            if nxt + 2 < bpc:
                efts[nxt + 2] = emit_eft(nxt + 2)
            for ui in range(4):
                emit_softmax_tile(bi, projT, eft, ui)
            if nxt < bpc:
                projs[nxt] = pproj.tile(
                    [128, 2, U], BF16, tag="projT", name=f"projT{nxt}"
                )
            for ui in range(4, 8):
                emit_softmax_tile(bi, projT, eft, ui)
                if nxt < bpc:
                    emit_mm1_group(uft_view(nxt), projs[nxt], ui - 4)

    # Runs Bacc.compile(): register allocation + event-semaphore splitting.
    nc.finalize()
    return nc


# Slot processing order over the width-sorted rank groups: widest first
# (overlaps the load ramp), then interleave narrow/wide so the ACT/DVE
# fixed-cost-bound narrow-slot softmax chains overlap wide-slot PE work
# instead of clustering at the end where the PE idles.
_SLOT_IL = [0, 7, 1, 6, 2, 5, 3, 4]


def _widths_for(num_enemy):
    ne = np.asarray(num_enemy).astype(np.int64)
    ne_eff = np.where(ne > 0, ne, N)
    order = np.argsort(-ne_eff, kind="stable")
    groups = order.reshape(BPC, N_CORES)  # rank group r, per core
    il = _SLOT_IL if BPC == len(_SLOT_IL) else list(range(BPC))
    groups = groups[il]  # slot k <- rank group il[k]
    slot_ne = ne_eff[groups]
    widths = tuple(
        int(max(128, -(-int(m) // 64) * 64)) for m in slot_ne.max(axis=1)
    )
    return groups, widths


def _prep_inputs(ufeat, efeat, num_enemy, v, g, b, widths):
    """Host prep: weight-norm, bf16 cast, mask poison, SBUF-layout staging."""
    ufeat = np.asarray(ufeat, dtype=np.float32)
    efeat = np.asarray(efeat, dtype=np.float32)
    num_enemy = np.asarray(num_enemy).astype(np.int64)
    v = np.asarray(v, dtype=np.float32)
    g = np.float32(np.asarray(g))
    b = np.asarray(b, dtype=np.float32)

    W = (g / np.float32(np.linalg.norm(v))) * v  # [E, K]
    wT = np.ascontiguousarray(W.T).astype(BF16_NP)  # [K, E]

    # SBUF layout: ufT[b, p, kt*U + u] = ufeat[b, u, kt*128+p]
    # [B, U, K] -> bf16 -> [B, K, U] -> [B, 4, 128, U] -> [B, 128, 4, U]
    ufT = (
        ufeat.astype(BF16_NP)
        .transpose(0, 2, 1)
        .reshape(B, 4, 128, U)
        .transpose(0, 2, 1, 3)
        .reshape(B, 128, 4 * U)
    )

    # efT in SBUF layout [B, 128, 2, N]: efT[b, p, et, n] = efeat[b, n, et*128+p]
    efT = efeat.astype(BF16_NP).transpose(0, 2, 1)  # [B, E, N]
    # Mask: poison masked efeat columns (n >= num_enemy) with -1e30. Since
    # proj >= 0 (relu) and a proj row is never identically 0 in practice,
    # masked logits land at <= -1e28 and exp underflows to exactly 0 — the
    # same 0 the reference's -1e9 bias produces. num_enemy==0 => all lanes
    # masked => uniform shift cancels in softmax => leave unpoisoned.
    ne = np.where(num_enemy > 0, num_enemy, N)
    col_masked = np.arange(N)[None, :] >= ne[:, None]  # [B, N]
    efT[np.broadcast_to(col_masked[:, None, :], efT.shape)] = BF16_NP(-1e30)
    efT = efT.reshape(B, 2, 128, N).transpose(0, 2, 1, 3)  # [B, 128, 2, N]

    return ufT, efT, wT, b


def _pack_ef(efT, perm, widths):
    """Per-core packed efeat: [128, sum(2*W)] per the compiled offsets."""
    cols = []
    for k, bi in enumerate(perm):
        w = widths[k]
        cols.append(efT[bi, :, :, :w].reshape(128, 2 * w))
    return np.ascontiguousarray(np.concatenate(cols, axis=1))


def _unpack_out(probc, widths):
    """[128, sum(8*W)] uint8 -> [bpc, U, N] f32 (q/255)."""
    bpc = len(widths)
    out = np.zeros((bpc, U, N), dtype=np.float32)
    off = 0
    for k, w in enumerate(widths):
        blk = probc[:, off : off + 8 * w].reshape(128, 8, w)
        # u = j*128 + p
        rows = blk.transpose(1, 0, 2).reshape(U, w).astype(np.float32)
        out[k, :, :w] = rows * np.float32(1.0 / 253.5)
        off += 8 * w
    return out


_nc_cache: dict[tuple, bass.Bass] = {}


def run(ufeat, efeat, num_enemy, v, g, b, trace=False):
    groups, widths = _widths_for(num_enemy)
    ufT, efT, wT, b = _prep_inputs(ufeat, efeat, num_enemy, v, g, b, widths)

    key = (BPC, widths)
    if key not in _nc_cache:
        _nc_cache[key] = _build_bass(BPC, widths)
    nc = _nc_cache[key]

    in_maps = []
    perms = []
    for c in range(N_CORES):
        perm = groups[:, c]  # batch index for each slot
        perms.append(perm)
        in_maps.append(
            {
                "ufT": np.ascontiguousarray(ufT[perm]),
                "efT": _pack_ef(efT, perm, widths),
                "wT": wT,
                "bias": b,
            }
        )

    res = run_bass_kernel_spmd(nc, in_maps, list(range(N_CORES)), trace=trace)
    out = np.empty((B, U, N), dtype=np.float32)
    for c in range(N_CORES):
        probc = np.asarray(res.results[c]["probc"])
        out[perms[c]] = _unpack_out(probc, widths)
    return out, res


def kernel(ufeat, efeat, num_enemy, v, g, b):
    out, _ = run(ufeat, efeat, num_enemy, v, g, b, trace=False)
    return out


# revision 18
# speedup vs baseline: 1.0403x; 1.0067x over previous
"""DotAttackHead kernel for Trainium2 (8 NeuronCores, data-parallel over batch).

prob = softmax(relu(ufeat @ W.T + b) @ efeat.T / sqrt(256) + mask_bias)
W = g * v / ||v||_F

Sharding: batch 64 -> 8 cores x 8 batches (data-parallel). Params replicated.

Host prep: weight-norm W, bf16 cast, mask folded into efeat (masked columns
poisoned with -1e30 so masked logits underflow exp to exactly 0), and BOTH
inputs staged in the exact SBUF tile layout ([p, kt, u] / [p, et, n]) so
every load is a pure linear 128-partition DMA with 4-8KB contiguous
per-partition chunks (the naive [K, U]-transposing load produces 1KB
descriptors and runs at ~1/4 line rate).

Device per batch b (software-pipelined across batches):
  mm1:  projT[e,u] = relu(wT.T @ ufT[b] + bias)   (PE bf16; bias+relu fused
        on DVE as tensor_scalar add/max reading PSUM, bf16 out)
  mm2:  psum[u,n]  = projT.T @ efT[b]             (PE bf16, fp32 PSUM)
  soft: et = Exp(psum/16) with accum_out row-sum (ACT, bf16, written
        directly into the ganged [128,4,W] store tile). The softmax
        division happens on the HOST during the f32 upcast (prob = et / s),
        which removes the reciprocal+multiply (35us of DVE) from the device.
Row sums gang into one [128, bpc, 8] tile, DMA'd once at the end.

Output is stored in a COMPACT partition-major layout: one flat bf16 tensor,
per (slot, 4-u-tile gang) a [128, 4, W] block written as one linear DMA
(4W*2B contiguous per partition -> max-size descriptors). The host
un-permutes to [U, N], divides by the row sums and zero-fills columns
[W, N). Masked-width specialization: batches sorted by effective width
descending, rank 8k+c -> (core c, slot k), slot width = slot max rounded
up to 128, compiled per-widths (NEFF-cached).

No max-subtraction: logits are O(+-6) so exp is safe in fp32, and softmax is
shift-invariant, so this matches the reference.

fp8 was evaluated for the matmuls (DoubleRow, 2x PE) and REJECTED: e4m3
quantization of proj+efeat pushes rel err to 4e-2 > the 2e-2 gate.
"""

from contextlib import ExitStack

import ml_dtypes
import numpy as np

import concourse.bass as bass
import concourse.mybir as mybir
import concourse.tile as tile
from concourse import bacc
from concourse.bass_utils import run_bass_kernel_spmd

N_CORES = 8
B = 64
U = 1024  # units
E = 256   # efeat dim
K = 512   # ufeat dim
N = 1024  # enemies
BPC = B // N_CORES  # batches per core

F32 = mybir.dt.float32
BF16 = mybir.dt.bfloat16
BF16_NP = ml_dtypes.bfloat16

def _build_bass(bpc: int = BPC, widths: tuple = ()) -> bass.Bass:
    if not widths:
        widths = (N,) * bpc
    assert len(widths) == bpc and all(w % 64 == 0 and 128 <= w <= N for w in widths)
    # Bacc (not raw Bass): its finalize() runs generate_event_semaphores,
    # which splits multi-wait instructions to satisfy TRN2's 1-wait limit.
    nc = bacc.Bacc(None, target_bir_lowering=False)

    # inputs staged host-side in SBUF layout: linear 128-partition loads
    ufT = nc.declare_dram_parameter("ufT", [bpc, 128, 4 * U], BF16, isOutput=False)
    efT_sizes = [2 * w for w in widths]
    ef_off = np.cumsum([0] + efT_sizes)
    efT = nc.declare_dram_parameter(
        "efT", [128, int(ef_off[-1])], BF16, isOutput=False
    )
    wT = nc.declare_dram_parameter("wT", [K, E], BF16, isOutput=False)
    bias = nc.declare_dram_parameter("bias", [E], F32, isOutput=False)
    # compact partition-major output: per (slot, gang) a [128, 4, W] block,
    # linear per partition. Host un-permutes + divides by row sums.
    pr_off = np.cumsum([0] + [8 * w for w in widths])  # per-slot elems/partition
    prob = nc.declare_dram_parameter(
        "probc", [128, int(pr_off[-1])], mybir.dt.uint8, isOutput=True
    )

    with tile.TileContext(nc) as tc, ExitStack() as ctx:
        singles = ctx.enter_context(tc.tile_pool(name="singles", bufs=1))
        pin = ctx.enter_context(tc.tile_pool(name="pin", bufs=4))
        pproj = ctx.enter_context(tc.tile_pool(name="pproj", bufs=3))
        pprob = ctx.enter_context(tc.tile_pool(name="pprob", bufs=3))
        pet = ctx.enter_context(tc.tile_pool(name="pet", bufs=4))
        psmall = ctx.enter_context(tc.tile_pool(name="psmall", bufs=16))
        pps1 = ctx.enter_context(tc.tile_pool(name="pps1", bufs=2, space="PSUM"))
        pps2 = ctx.enter_context(tc.tile_pool(name="pps2", bufs=3, space="PSUM"))

        # ---- resident constants (issued after the first uft half) ----
        wt_sb = singles.tile([128, 4, E], BF16)
        b_sb = singles.tile([128, 2], F32)

        def emit_consts():
            # consts ride the scalar HWDGE ring: they land in parallel with
            # uft0-h0 on the sync ring instead of queueing behind it
            # wT as 4 k-tiles: wt_sb[p, kt, e] = wT[kt*128+p, e]
            nc.scalar.dma_start(
                out=wt_sb, in_=wT[:, :].rearrange("(kt p) e -> p kt e", p=128)
            )
            # bias as 2 e-tiles on partitions: b_sb[p, et] = bias[et*128+p]
            nc.scalar.dma_start(
                out=b_sb, in_=bias[:].rearrange("(et p) -> p et", p=128)
            )

        def emit_uft_pair(bi):
            # one [128, 2, 4, U] tile holds batches (bi, bi+1): fewer, bigger
            # DMAs (16KB/partition) -> fewer ring slots + completion sems
            uftp = pin.tile([128, 2, 4, U], BF16, tag="uftp", name=f"uftp{bi}")
            if bi == 0:
                # ramp: kt0 of u-half 0 first (128KB) so the very first
                # matmul's DMA-completion wait resolves ~2us sooner; then
                # the rest of half 0, then half 1
                nc.sync.dma_start(
                    out=uftp[:, 0, 0:1, 0:512],
                    in_=ufT[0, :, :].rearrange("p (kt u) -> p kt u", kt=4)[
                        :, 0:1, 0:512
                    ],
                )
                emit_consts()
                nc.sync.dma_start(
                    out=uftp[:, 0, 1:4, 0:512],
                    in_=ufT[0, :, :].rearrange("p (kt u) -> p kt u", kt=4)[
                        :, 1:4, 0:512
                    ],
                )
                nc.sync.dma_start(
                    out=uftp[:, 0, :, 512:1024],
                    in_=ufT[0, :, :].rearrange("p (kt u) -> p kt u", kt=4)[
                        :, :, 512:1024
                    ],
                )
                if bpc > 1:
                    nc.sync.dma_start(
                        out=uftp[:, 1],
                        in_=ufT[1, :, :].rearrange("p (kt u) -> p kt u", kt=4),
                    )
            else:
                nc.sync.dma_start(
                    out=uftp,
                    in_=ufT[bi : bi + 2, :, :].rearrange(
                        "b p (kt u) -> p b kt u", kt=4
                    ),
                )
            return uftp

        def emit_eft(bi, eng=None):
            # the first two slots' efeat ride the (idle-at-start) scalar
            # ring so mm2 of slot 0 isn't gated behind 2MB of uft loads
            eng = eng or (nc.scalar if bi < 2 else nc.sync)
            W = widths[bi]
            eft = pin.tile([128, 2, W], BF16, tag="eft", name=f"eft{bi}")
            eng.dma_start(
                out=eft,
                in_=efT[:, int(ef_off[bi]) : int(ef_off[bi + 1])].rearrange(
                    "p (et n) -> p et n", et=2
                ),
            )
            return eft

        def emit_mm1_group(uft, projT, gi):
            # group gi -> (ej, uc), uc-major: both e-halves of u-chunk 0 come
            # first, so mm2 tiles u0..u3 unblock after 2 groups instead of 4
            ej, uc = gi % 2, gi // 2
            esl = slice(ej * 128, (ej + 1) * 128)
            usl = slice(uc * 512, (uc + 1) * 512)
            ps1 = pps1.tile([128, 512], F32, tag="ps1")
            for kj in range(4):
                nc.tensor.matmul(
                    ps1,
                    lhsT=wt_sb[:, kj, esl],
                    rhs=uft[:, kj, usl],
                    start=(kj == 0),
                    stop=(kj == 3),
                )
            # relu(x + b) = max(x + b, 0) fused on DVE; casts to bf16
            nc.vector.tensor_scalar(
                out=projT[:, ej, usl],
                in0=ps1,
                scalar1=b_sb[:, ej : ej + 1],
                scalar2=0.0,
                op0=mybir.AluOpType.add,
                op1=mybir.AluOpType.max,
            )

        pair_state = {}

        def emit_softmax_tile(bi, projT, eft, ui):
            # only the first widths[bi] columns are live
            W = widths[bi]
            nslices = [slice(0, min(512, W))] + ([slice(512, W)] if W > 512 else [])
            uslice = slice(ui * 128, (ui + 1) * 128)
            ps2 = pps2.tile([128, W], F32, tag="ps2", name=f"ps2_{bi}_{ui}")
            # e-major: consecutive matmuls share the same lhsT (weight reuse)
            for ej in range(2):
                for nsl in nslices:
                    nc.tensor.matmul(
                        ps2[:, nsl],
                        lhsT=projT[:, ej, uslice],
                        rhs=eft[:, ej, nsl],
                        start=(ej == 0),
                        stop=(ej == 1),
                    )
            # gang ALL 8 u-tiles of the slot into one [128, 8, W] uint8
            # store tile: one output DMA per slot (8 total)
            if ui == 0:
                pair_state["tile"] = pprob.tile(
                    [128, 8, W], mybir.dt.uint8, tag="prob", name=f"prob{bi}"
                )
            prob_t = pair_state["tile"]
            et = pet.tile([128, W], BF16, tag="et", name=f"et{bi}_{ui}")
            s = psmall.tile([128, 1], F32, tag="s")
            nc.scalar.activation(
                out=et,
                in_=ps2,
                func=mybir.ActivationFunctionType.Exp,
                scale=1.0 / 16.0,
                accum_out=s,
            )
            r = psmall.tile([128, 1], F32, tag="r")
            nc.vector.reciprocal(out=r, in_=s)
            # q = (et * r) * QSCALE -> uint8. QSCALE = 253.5 (not 255) so
            # bf16 jitter on a prob ~= 1.0 can never push q past 255 (numpy
            # wraps, HW saturates -- rely on neither). Conversion truncates
            # (or rounds, HW-dependent): either way abs error <= 1 LSB =
            # 3.9e-3, under the ~1.2e-2 absolute budget, and the output
            # stream halves vs bf16. Host divides by QSCALE.
            nc.vector.tensor_scalar(
                out=prob_t[:, ui, :],
                in0=et,
                scalar1=r,
                scalar2=253.5,
                op0=mybir.AluOpType.mult,
                op1=mybir.AluOpType.mult,
            )
            if ui == 7:
                off = int(pr_off[bi])
                # stores alternate between the two HWDGE rings (SP / ACT):
                # the store direction pays HBM-write receipts and runs at
                # ~half line rate per ring, so two rings drain in parallel
                eng = nc.sync if bi % 2 == 0 else nc.scalar
                eng.dma_start(
                    out=prob[:, off : off + 8 * W].rearrange(
                        "p (j n) -> p j n", j=8
                    ),
                    in_=prob_t,
                )

        # Software-pipelined emission: mm1 groups for batch bi+1 are emitted
        # between softmax tiles of batch bi's second half, so the PE never
        # monopolizes a contiguous ~4us window on mm1 while ACT's 3-deep
        # PSUM backlog drains.
        pairs = {0: emit_uft_pair(0)}
        efts = {bi: emit_eft(bi) for bi in range(min(3, bpc))}
        projs = {0: pproj.tile([128, 2, U], BF16, tag="projT", name="projT0")}

        def uft_view(bi):
            return pairs[bi - bi % 2][:, bi % 2]

        for gi in range(4):
            emit_mm1_group(uft_view(0), projs[0], gi)
        for bi in range(bpc):
            eft = efts[bi]
            projT = projs[bi]
            nxt = bi + 1
            if nxt < bpc and nxt % 2 == 0:
                pairs[nxt] = emit_uft_pair(nxt)
            if nxt + 2 < bpc:
                efts[nxt + 2] = emit_eft(nxt + 2)
            for ui in range(4):
                emit_softmax_tile(bi, projT, eft, ui)
            if nxt < bpc:
                projs[nxt] = pproj.tile(
                    [128, 2, U], BF16, tag="projT", name=f"projT{nxt}"
                )
            for ui in range(4, 8):
                emit_softmax_tile(bi, projT, eft, ui)
                if nxt < bpc:
                    emit_mm1_group(uft_view(nxt), projs[nxt], ui - 4)

    # Runs Bacc.compile(): register allocation + event-semaphore splitting.
    nc.finalize()
    return nc


def _widths_for(num_enemy):
    """Slot k <- width-rank group k (descending): widest slot first
    (overlaps the load ramp), narrowest last (short drain tail)."""
    ne = np.asarray(num_enemy).astype(np.int64)
    ne_eff = np.where(ne > 0, ne, N)
    order = np.argsort(-ne_eff, kind="stable")
    groups = order.reshape(BPC, N_CORES)  # slot k, per core
    slot_ne = ne_eff[groups]
    widths = tuple(
        int(max(128, -(-int(m) // 64) * 64)) for m in slot_ne.max(axis=1)
    )
    return groups, widths


def _prep_inputs(ufeat, efeat, num_enemy, v, g, b, widths):
    """Host prep: weight-norm, bf16 cast, mask poison, SBUF-layout staging."""
    ufeat = np.asarray(ufeat, dtype=np.float32)
    efeat = np.asarray(efeat, dtype=np.float32)
    num_enemy = np.asarray(num_enemy).astype(np.int64)
    v = np.asarray(v, dtype=np.float32)
    g = np.float32(np.asarray(g))
    b = np.asarray(b, dtype=np.float32)

    W = (g / np.float32(np.linalg.norm(v))) * v  # [E, K]
    wT = np.ascontiguousarray(W.T).astype(BF16_NP)  # [K, E]

    # SBUF layout: ufT[b, p, kt*U + u] = ufeat[b, u, kt*128+p]
    # [B, U, K] -> bf16 -> [B, K, U] -> [B, 4, 128, U] -> [B, 128, 4, U]
    ufT = (
        ufeat.astype(BF16_NP)
        .transpose(0, 2, 1)
        .reshape(B, 4, 128, U)
        .transpose(0, 2, 1, 3)
        .reshape(B, 128, 4 * U)
    )

    # efT in SBUF layout [B, 128, 2, N]: efT[b, p, et, n] = efeat[b, n, et*128+p]
    efT = efeat.astype(BF16_NP).transpose(0, 2, 1)  # [B, E, N]
    # Mask: poison masked efeat columns (n >= num_enemy) with -1e30. Since
    # proj >= 0 (relu) and a proj row is never identically 0 in practice,
    # masked logits land at <= -1e28 and exp underflows to exactly 0 — the
    # same 0 the reference's -1e9 bias produces. num_enemy==0 => all lanes
    # masked => uniform shift cancels in softmax => leave unpoisoned.
    ne = np.where(num_enemy > 0, num_enemy, N)
    col_masked = np.arange(N)[None, :] >= ne[:, None]  # [B, N]
    efT[np.broadcast_to(col_masked[:, None, :], efT.shape)] = BF16_NP(-1e30)
    efT = efT.reshape(B, 2, 128, N).transpose(0, 2, 1, 3)  # [B, 128, 2, N]

    return ufT, efT, wT, b


def _pack_ef(efT, perm, widths):
    """Per-core packed efeat: [128, sum(2*W)] per the compiled offsets."""
    cols = []
    for k, bi in enumerate(perm):
        w = widths[k]
        cols.append(efT[bi, :, :, :w].reshape(128, 2 * w))
    return np.ascontiguousarray(np.concatenate(cols, axis=1))


def _unpack_out(probc, widths):
    """[128, sum(8*W)] uint8 -> [bpc, U, N] f32 (q/255)."""
    bpc = len(widths)
    out = np.zeros((bpc, U, N), dtype=np.float32)
    off = 0
    for k, w in enumerate(widths):
        blk = probc[:, off : off + 8 * w].reshape(128, 8, w)
        # u = j*128 + p
        rows = blk.transpose(1, 0, 2).reshape(U, w).astype(np.float32)
        out[k, :, :w] = rows * np.float32(1.0 / 253.5)
        off += 8 * w
    return out


_nc_cache: dict[tuple, bass.Bass] = {}


def run(ufeat, efeat, num_enemy, v, g, b, trace=False):
    groups, widths = _widths_for(num_enemy)
    ufT, efT, wT, b = _prep_inputs(ufeat, efeat, num_enemy, v, g, b, widths)

    key = (BPC, widths)
    if key not in _nc_cache:
        _nc_cache[key] = _build_bass(BPC, widths)
    nc = _nc_cache[key]

    in_maps = []
    perms = []
    for c in range(N_CORES):
        perm = groups[:, c]  # batch index for each slot
        perms.append(perm)
        in_maps.append(
            {
                "ufT": np.ascontiguousarray(ufT[perm]),
                "efT": _pack_ef(efT, perm, widths),
                "wT": wT,
                "bias": b,
            }
        )

    res = run_bass_kernel_spmd(nc, in_maps, list(range(N_CORES)), trace=trace)
    out = np.empty((B, U, N), dtype=np.float32)
    for c in range(N_CORES):
        probc = np.asarray(res.results[c]["probc"])
        out[perms[c]] = _unpack_out(probc, widths)
    return out, res


def kernel(ufeat, efeat, num_enemy, v, g, b):
    out, _ = run(ufeat, efeat, num_enemy, v, g, b, trace=False)
    return out
